# revision 1
# baseline (speedup 1.0000x reference)
"""GATv2 (3 layers, heads=1) + global mean pool + linear on 8 Trainium2 cores.

Sharding: edges partitioned by dst-node range (8 x 6250 nodes). Per core,
local nodes are degree-sorted into 49 chunks of 128 rows; each chunk is
padded to a uniform per-chunk degree so segmented softmax / scatter-add
become dense row-wise ops. Node feature transforms (h@Wl, h@Wr) are
replicated on every core; per-edge source features come from an indirect
(gather) DMA out of DRAM. h is carried transposed ([64, nodes]) between
layers via AllGather so the next layer's matmuls need no on-device
transposes of the full node set.
"""

import os
import sys
from contextlib import ExitStack
from dataclasses import dataclass

import numpy as np

for _p in ("/opt/trn_rl_repo", "/root/.axon_site/_ro/trn_rl_repo"):
    if os.path.isdir(_p) and _p not in sys.path:
        sys.path.insert(0, _p)

import concourse.bass as bass
import concourse.tile as tile
from concourse import bacc, mybir
from concourse.bass import IndirectOffsetOnAxis
from concourse.masks import make_identity

F32 = mybir.dt.float32
I32 = mybir.dt.int32
ALU = mybir.AluOpType
ACT = mybir.ActivationFunctionType
AX = mybir.AxisListType


@dataclass
class Cfg:
    N: int          # real nodes
    E: int
    G: int          # graphs
    C: int = 8      # cores
    D: int = 64     # feature dim
    NPC: int = 0    # real nodes per core
    KCH: int = 0    # chunks of 128 rows per core
    NPCP: int = 0   # padded nodes per core = KCH*128
    NT: int = 0     # C*NPCP

    def __post_init__(self):
        self.NPC = self.N // self.C
        self.KCH = (self.NPC + 127) // 128
        self.NPCP = self.KCH * 128
        self.NT = self.C * self.NPCP


def host_prep(cfg, x, edge_index, edge_attr, batch, weights):
    C, NPC, NPCP, KCH, D = cfg.C, cfg.NPC, cfg.NPCP, cfg.KCH, cfg.D
    N, E = cfg.N, cfg.E
    src = np.asarray(edge_index[0], np.int64)
    dst = np.asarray(edge_index[1], np.int64)
    ea = np.asarray(edge_attr, np.float32).reshape(-1)
    deg = np.bincount(dst, minlength=N)

    pid_of = np.empty(N, np.int64)
    deg_sorted = np.zeros((C, NPCP), np.int64)
    for c in range(C):
        lo = c * NPC
        order = np.argsort(-deg[lo:lo + NPC], kind="stable") + lo
        pid_of[order] = c * NPCP + np.arange(NPC)
        deg_sorted[c, :NPC] = deg[order]

    # per-chunk padded degree (max over cores; rows are degree-sorted desc,
    # so the first row of a chunk is its max)
    Dpad = np.zeros(KCH, np.int64)
    for k in range(KCH):
        m = int(deg_sorted[:, k * 128].max())
        Dpad[k] = max(4, ((m + 3) // 4) * 4)
    offs = np.zeros(KCH + 1, np.int64)
    offs[1:] = np.cumsum(Dpad)
    S = int(offs[-1])

    # slot fill (fully vectorized)
    e_ord = np.argsort(dst, kind="stable")
    dst_s, src_s, ea_s = dst[e_ord], src[e_ord], ea[e_ord]
    startn = np.zeros(N + 1, np.int64)
    startn[1:] = np.cumsum(deg)
    j = np.arange(E) - startn[dst_s]
    gpid = pid_of[dst_s]
    c_a = gpid // NPCP
    loc = gpid % NPCP
    k_a = loc // 128
    r_a = loc % 128
    col = offs[k_a] + j
    srcs = np.zeros((C, 128, S), np.int32)
    eas = np.zeros((C, 128, S), np.float32)
    lbs = np.full((C, 128, S), -1e9, np.float32)
    srcs[c_a, r_a, col] = pid_of[src_s].astype(np.int32)
    eas[c_a, r_a, col] = ea_s
    lbs[c_a, r_a, col] = 0.0

    # transposed, permuted node features [C, 64, NPCP]
    x_P = np.zeros((cfg.NT, D), np.float32)
    x_P[pid_of] = np.asarray(x, np.float32)
    xT = np.ascontiguousarray(x_P.reshape(C, NPCP, D).transpose(0, 2, 1))

    # xr gather ids per core [128, KCH]
    tids = np.zeros((C, 128, KCH), np.int32)
    r = np.arange(128)
    for c in range(C):
        for k in range(KCH):
            tids[c, :, k] = c * NPCP + k * 128 + r

    # pooling indicator [C, 128, KCH*G64]; G padded to 64 cols for matmul
    GP = 64
    pool = np.zeros((cfg.NT, GP), np.float32)
    pool[pid_of, np.asarray(batch, np.int64)] = 1.0
    pind = np.ascontiguousarray(
        pool.reshape(C, KCH, 128, GP).transpose(0, 2, 1, 3).reshape(C, 128, KCH * GP))

    wlr = np.stack([np.hstack([weights[f"Wl{l}"], weights[f"Wr{l}"]]).astype(np.float32)
                    for l in (1, 2, 3)])                      # [3, 64, 128]
    wer = np.stack([np.repeat(weights[f"We{l}"].astype(np.float32), 128, axis=0)
                    for l in (1, 2, 3)])                      # [3, 128, 64]
    attr = np.stack([np.tile(weights[f"att{l}"].astype(np.float32), (128, 1))
                     for l in (1, 2, 3)])                     # [3, 128, 64]
    # biases (zero in the reference init, but handle generally)
    blbr = np.stack([np.tile(np.concatenate([weights[f"bl{l}"], weights[f"br{l}"]]).astype(np.float32), (128, 1))
                     for l in (1, 2, 3)])                     # [3, 128, 128]
    brep = np.stack([np.tile(weights[f"b{l}"].astype(np.float32), (128, 1))
                     for l in (1, 2, 3)])                     # [3, 128, 64]
    has_blbr = bool(np.any(blbr))
    has_b = bool(np.any(brep))

    counts = np.bincount(np.asarray(batch, np.int64), minlength=cfg.G).astype(np.float32)
    assert deg.min() >= 1, "isolated real node: unsupported fast path"

    meta = dict(Dpad=Dpad, offs=offs, S=S, has_blbr=has_blbr, has_b=has_b)
    percore = [dict(srcs=srcs[c], eas=eas[c], lbs=lbs[c], tids=tids[c], pind=pind[c],
                    xT=xT, wlr=wlr, wer=wer, attr=attr, blbr=blbr, brep=brep)
               for c in range(C)]
    return meta, percore, counts


def build_program(cfg, meta, dbg=False):
    Dpad, offs, S = meta["Dpad"], meta["offs"], meta["S"]
    C, D, KCH, NPCP, NT = cfg.C, cfg.D, cfg.KCH, cfg.NPCP, cfg.NT
    nc = bacc.Bacc("TRN2", target_bir_lowering=False, debug=False,
                   enable_asserts=False, num_devices=C)

    xT_h = nc.dram_tensor("xT", [C, D, NPCP], F32, kind="ExternalInput")
    srcs_h = nc.dram_tensor("srcs", [128, S], I32, kind="ExternalInput")
    eas_h = nc.dram_tensor("eas", [128, S], F32, kind="ExternalInput")
    lbs_h = nc.dram_tensor("lbs", [128, S], F32, kind="ExternalInput")
    tids_h = nc.dram_tensor("tids", [128, KCH], I32, kind="ExternalInput")
    pind_h = nc.dram_tensor("pind", [128, KCH * 64], F32, kind="ExternalInput")
    wlr_h = nc.dram_tensor("wlr", [3, 64, 128], F32, kind="ExternalInput")
    wer_h = nc.dram_tensor("wer", [3, 128, 64], F32, kind="ExternalInput")
    attr_h = nc.dram_tensor("attr", [3, 128, 64], F32, kind="ExternalInput")
    blbr_h = nc.dram_tensor("blbr", [3, 128, 128], F32, kind="ExternalInput")
    brep_h = nc.dram_tensor("brep", [3, 128, 64], F32, kind="ExternalInput")
    pout_h = nc.dram_tensor("pool_part", [64, 64], F32, kind="ExternalOutput")
    if dbg:
        xl_d = nc.dram_tensor("xl_dump", [NT, D], F32, kind="ExternalOutput")
        hTP_d = nc.dram_tensor("hTP_dump", [C, D, NPCP], F32, kind="ExternalOutput")
        g_d = nc.dram_tensor("g_dump", [128, int(Dpad[0]) * 64], F32, kind="ExternalOutput")

    xl_h = nc.dram_tensor("xl_scr", [NT, D], F32, kind="Internal")
    xr_h = nc.dram_tensor("xr_scr", [NT, D], F32, kind="Internal")
    hsh_h = nc.dram_tensor("hsh", [D, NPCP], F32, kind="Internal")
    hTP_h = nc.dram_tensor("hTP", [C, D, NPCP], F32, kind="Internal",
                           addr_space="Shared")

    has_blbr, has_b = meta["has_blbr"], meta["has_b"]

    with ExitStack() as ctx:
        tc = ctx.enter_context(tile.TileContext(nc))
        cpool = ctx.enter_context(tc.tile_pool(name="const", bufs=1))
        tpool = ctx.enter_context(tc.tile_pool(name="lhsT", bufs=2))
        xpool = ctx.enter_context(tc.tile_pool(name="xx", bufs=2))
        gpool = ctx.enter_context(tc.tile_pool(name="edge_g", bufs=2))
        t1pool = ctx.enter_context(tc.tile_pool(name="edge_t1", bufs=2))
        t2pool = ctx.enter_context(tc.tile_pool(name="edge_t2", bufs=2))
        spool = ctx.enter_context(tc.tile_pool(name="small", bufs=8))
        opool = ctx.enter_context(tc.tile_pool(name="out", bufs=4))
        mmpool = ctx.enter_context(tc.tile_pool(name="psum_mm", bufs=2, space="PSUM"))
        tppool = ctx.enter_context(tc.tile_pool(name="psum_tp", bufs=2, space="PSUM"))
        pppool = ctx.enter_context(tc.tile_pool(name="psum_pool", bufs=1, space="PSUM"))

        # resident SBUF
        ident = cpool.tile([128, 128], F32)
        make_identity(nc, ident[:])
        srcs_sb = cpool.tile([128, S], I32)
        nc.sync.dma_start(srcs_sb[:], srcs_h.ap())
        eas_sb = cpool.tile([128, S], F32)
        nc.sync.dma_start(eas_sb[:], eas_h.ap())
        lbs_sb = cpool.tile([128, S], F32)
        nc.sync.dma_start(lbs_sb[:], lbs_h.ap())
        tids_sb = cpool.tile([128, KCH], I32)
        nc.sync.dma_start(tids_sb[:], tids_h.ap())
        pind_sb = cpool.tile([128, KCH * 64], F32)
        nc.sync.dma_start(pind_sb[:], pind_h.ap())
        wlr_sb, wer_sb, attr_sb, blbr_sb, brep_sb = [], [], [], [], []
        for l in range(3):
            w1 = cpool.tile([64, 128], F32)
            nc.sync.dma_start(w1[:], wlr_h.ap()[l])
            wlr_sb.append(w1)
            w2 = cpool.tile([128, 64], F32)
            nc.sync.dma_start(w2[:], wer_h.ap()[l])
            wer_sb.append(w2)
            w3 = cpool.tile([128, 64], F32)
            nc.sync.dma_start(w3[:], attr_h.ap()[l])
            attr_sb.append(w3)
            if has_blbr:
                w4 = cpool.tile([128, 128], F32)
                nc.sync.dma_start(w4[:], blbr_h.ap()[l])
                blbr_sb.append(w4)
            if has_b:
                w5 = cpool.tile([128, 64], F32)
                nc.sync.dma_start(w5[:], brep_h.ap()[l])
                brep_sb.append(w5)

        packs = [4] * (KCH // 4) + ([KCH % 4] if KCH % 4 else [])

        pp = None
        for l in range(3):
            src3d = xT_h.ap() if l == 0 else hTP_h.ap()
            # ---- transform: xl/xr = h @ [Wl|Wr] for ALL nodes (replicated)
            for sec in range(C):
                t0 = 0
                for gsz in packs:
                    lh = tpool.tile([64, gsz * 128], F32)
                    nc.sync.dma_start(lh[:], src3d[sec, :, t0 * 128:(t0 + gsz) * 128])
                    ps = mmpool.tile([128, gsz * 128], F32)
                    for a in range(gsz):
                        nc.tensor.matmul(out=ps[:, a * 128:(a + 1) * 128],
                                         lhsT=lh[:, a * 128:(a + 1) * 128],
                                         rhs=wlr_sb[l][:], start=True, stop=True)
                    xx = xpool.tile([128, gsz * 128], F32)
                    if has_blbr:
                        bb = blbr_sb[l][:].unsqueeze(1).to_broadcast([128, gsz, 128])
                        nc.vector.tensor_tensor(
                            out=xx[:].rearrange("p (a q) -> p a q", q=128),
                            in0=ps[:].rearrange("p (a q) -> p a q", q=128),
                            in1=bb, op=ALU.add)
                    else:
                        nc.scalar.activation(out=xx[:], in_=ps[:], func=ACT.Copy)
                    row0 = sec * NPCP + t0 * 128
                    xx3 = xx[:].rearrange("p (a q) -> p a q", q=128)
                    dst_l = xl_h.ap()[row0:row0 + gsz * 128, :].rearrange(
                        "(a p) d -> p a d", p=128)
                    dst_r = xr_h.ap()[row0:row0 + gsz * 128, :].rearrange(
                        "(a p) d -> p a d", p=128)
                    nc.sync.dma_start(dst_l, xx3[:, :, 0:64])
                    nc.sync.dma_start(dst_r, xx3[:, :, 64:128])
                    t0 += gsz

            # ---- edge stage over chunks of 128 dst nodes
            if l == 2:
                pp = pppool.tile([64, 64], F32)
            for k in range(KCH):
                Dk = int(Dpad[k])
                off = int(offs[k])
                g = gpool.tile([128, Dk * 64], F32)
                g3 = g[:].rearrange("p (s d) -> p s d", d=64)
                for s in range(Dk):
                    nc.gpsimd.indirect_dma_start(
                        out=g[:, s * 64:(s + 1) * 64], out_offset=None, in_=xl_h.ap(),
                        in_offset=IndirectOffsetOnAxis(
                            ap=srcs_sb[:, off + s:off + s + 1], axis=0))
                if dbg and l == 0 and k == 0:
                    nc.sync.dma_start(g_d.ap(), g[:])
                xr_t = opool.tile([128, 64], F32)
                nc.gpsimd.indirect_dma_start(
                    out=xr_t[:], out_offset=None, in_=xr_h.ap(),
                    in_offset=IndirectOffsetOnAxis(ap=tids_sb[:, k:k + 1], axis=0))

                t1 = t1pool.tile([128, Dk * 64], F32)
                t1v = t1[:].rearrange("p (s d) -> p s d", d=64)
                eav = eas_sb[:, off:off + Dk].unsqueeze(2).to_broadcast([128, Dk, 64])
                wv = wer_sb[l][:].unsqueeze(1).to_broadcast([128, Dk, 64])
                nc.vector.tensor_tensor(out=t1v, in0=eav, in1=wv, op=ALU.mult)
                nc.vector.tensor_tensor(out=t1v, in0=t1v, in1=g3, op=ALU.add)
                xv = xr_t[:].unsqueeze(1).to_broadcast([128, Dk, 64])
                nc.vector.tensor_tensor(out=t1v, in0=t1v, in1=xv, op=ALU.add)
                # leaky relu: max(z, 0.2 z)
                t2 = t2pool.tile([128, Dk * 64], F32)
                t2v = t2[:].rearrange("p (s d) -> p s d", d=64)
                nc.scalar.activation(out=t2[:], in_=t1[:], func=ACT.Copy, scale=0.2)
                nc.vector.tensor_tensor(out=t2[:], in0=t2[:], in1=t1[:], op=ALU.max)
                av = attr_sb[l][:].unsqueeze(1).to_broadcast([128, Dk, 64])
                nc.vector.tensor_tensor(out=t1v, in0=t2v, in1=av, op=ALU.mult)
                lg = spool.tile([128, Dk], F32)
                nc.vector.tensor_reduce(out=lg[:], in_=t1v, axis=AX.X, op=ALU.add)
                nc.vector.tensor_tensor(out=lg[:], in0=lg[:],
                                        in1=lbs_sb[:, off:off + Dk], op=ALU.add)
                mn = spool.tile([128, 1], F32)
                nc.vector.tensor_reduce(out=mn[:], in_=lg[:], axis=AX.X, op=ALU.max,
                                        negate=True)
                pt = spool.tile([128, Dk], F32)
                nc.scalar.activation(out=pt[:], in_=lg[:], func=ACT.Exp,
                                     bias=mn[:, 0:1], scale=1.0)
                st = spool.tile([128, 1], F32)
                nc.vector.tensor_reduce(out=st[:], in_=pt[:], axis=AX.X, op=ALU.add)
                ri = spool.tile([128, 1], F32)
                nc.vector.reciprocal(ri[:], st[:])
                pv = pt[:].unsqueeze(2).to_broadcast([128, Dk, 64])
                nc.vector.tensor_tensor(out=t2v, in0=g3, in1=pv, op=ALU.mult)
                cur = Dk
                while cur > 1:
                    h = cur // 2
                    nc.vector.tensor_tensor(out=t2v[:, 0:h, :], in0=t2v[:, 0:h, :],
                                            in1=t2v[:, h:2 * h, :], op=ALU.add)
                    if cur % 2:
                        nc.vector.tensor_tensor(out=t2v[:, 0:1, :], in0=t2v[:, 0:1, :],
                                                in1=t2v[:, cur - 1:cur, :], op=ALU.add)
                    cur = h
                ot = opool.tile([128, 64], F32)
                riv = ri[:].to_broadcast([128, 64])
                nc.vector.tensor_tensor(out=ot[:], in0=t2[:, 0:64], in1=riv,
                                        op=ALU.mult)
                if has_b:
                    nc.vector.tensor_tensor(out=ot[:], in0=ot[:], in1=brep_sb[l][:],
                                            op=ALU.add)
                if l < 2:
                    ot2 = opool.tile([128, 64], F32)
                    nc.scalar.activation(out=ot2[:], in_=ot[:], func=ACT.Relu)
                    tp = tppool.tile([64, 128], F32)
                    nc.tensor.transpose(out=tp[:], in_=ot2[:], identity=ident[:])
                    hT_t = opool.tile([64, 128], F32)
                    nc.scalar.activation(out=hT_t[:], in_=tp[:], func=ACT.Copy)
                    nc.sync.dma_start(hsh_h.ap()[:, k * 128:(k + 1) * 128], hT_t[:])
                else:
                    nc.tensor.matmul(out=pp[:], lhsT=pind_sb[:, k * 64:(k + 1) * 64],
                                     rhs=ot[:], start=(k == 0), stop=(k == KCH - 1))
            if l < 2:
                nc.gpsimd.collective_compute(
                    "AllGather", ALU.bypass,
                    replica_groups=[list(range(C))],
                    ins=[hsh_h.ap().opt()], outs=[hTP_h.ap().opt()])
            if dbg and l == 0:
                nc.sync.dma_start(xl_d.ap(), xl_h.ap())
                nc.sync.dma_start(hTP_d.ap(), hTP_h.ap())

        po = opool.tile([64, 64], F32)
        nc.vector.tensor_copy(po[:], pp[:])
        nc.sync.dma_start(pout_h.ap(), po[:])

    nc.compile()
    return nc


_CACHE = {}


def _get_weights(inputs):
    keys = []
    for l in (1, 2, 3):
        keys += [f"Wl{l}", f"bl{l}", f"Wr{l}", f"br{l}", f"We{l}", f"att{l}", f"b{l}"]
    return {k: np.asarray(inputs[k], np.float32) for k in keys}


def run_raw(inputs, trace=False):
    from concourse import bass_utils

    x = np.asarray(inputs["x"], np.float32)
    edge_index = np.asarray(inputs["edge_index"])
    edge_attr = np.asarray(inputs["edge_attr"], np.float32)
    batch = np.asarray(inputs["batch"])
    N, E = x.shape[0], edge_index.shape[1]
    G = 64
    cfg = Cfg(N=N, E=E, G=G)
    weights = _get_weights(inputs)

    meta, percore, counts = host_prep(cfg, x, edge_index, edge_attr, batch, weights)

    key = (N, E, int(meta["S"]), tuple(int(d) for d in meta["Dpad"]),
           meta["has_blbr"], meta["has_b"])
    if key not in _CACHE:
        _CACHE[key] = build_program(cfg, meta)
    nc = _CACHE[key]

    in_maps = [dict(pc) for pc in percore]
    res = bass_utils.run_bass_kernel_spmd(nc, in_maps, core_ids=list(range(cfg.C)),
                                          trace=trace)
    parts = np.zeros((64, 64), np.float64)
    for c in range(cfg.C):
        parts += np.asarray(res.results[c]["pool_part"], np.float64)
    hG = parts[:G, :cfg.D] / np.maximum(counts, 1.0)[:, None]
    Wlin = np.asarray(inputs["Wlin"], np.float64)
    blin = np.asarray(inputs["blin"], np.float64)
    return (hG @ Wlin + blin).astype(np.float32), res


def kernel(**inputs):
    out, _ = run_raw(inputs, trace=False)
    return out



# revision 2
# speedup vs baseline: 1.5032x; 1.5032x over previous
"""GATv2 (3 layers, heads=1) + global mean pool + linear on 8 Trainium2 cores.

Sharding: edges partitioned by dst-node range (8 x 6250 nodes). Per core,
local nodes are degree-sorted into 49 chunks of 128 rows; each chunk is
padded to a uniform per-chunk degree so segmented softmax / scatter-add
become dense row-wise ops. Node feature transforms (h@Wl, h@Wr) are
replicated on every core; per-edge source features come from an indirect
(gather) DMA out of DRAM. h is carried transposed ([64, nodes]) between
layers via AllGather so the next layer's matmuls need no on-device
transposes of the full node set.
"""

import os
import sys
from contextlib import ExitStack
from dataclasses import dataclass

import numpy as np

for _p in ("/opt/trn_rl_repo", "/root/.axon_site/_ro/trn_rl_repo"):
    if os.path.isdir(_p) and _p not in sys.path:
        sys.path.insert(0, _p)

import concourse.bass as bass
import concourse.tile as tile
from concourse import bacc, mybir
from concourse.bass import IndirectOffsetOnAxis
from concourse.masks import make_identity

F32 = mybir.dt.float32
I32 = mybir.dt.int32
ALU = mybir.AluOpType
ACT = mybir.ActivationFunctionType
AX = mybir.AxisListType


@dataclass
class Cfg:
    N: int          # real nodes
    E: int
    G: int          # graphs
    C: int = 8      # cores
    D: int = 64     # feature dim
    NPC: int = 0    # real nodes per core
    KCH: int = 0    # chunks of 128 rows per core
    NPCP: int = 0   # padded nodes per core = KCH*128
    NT: int = 0     # C*NPCP

    def __post_init__(self):
        self.NPC = self.N // self.C
        self.KCH = (self.NPC + 127) // 128
        self.NPCP = self.KCH * 128
        self.NT = self.C * self.NPCP


def host_prep(cfg, x, edge_index, edge_attr, batch, weights):
    C, NPC, NPCP, KCH, D = cfg.C, cfg.NPC, cfg.NPCP, cfg.KCH, cfg.D
    N, E = cfg.N, cfg.E
    src = np.asarray(edge_index[0], np.int64)
    dst = np.asarray(edge_index[1], np.int64)
    ea = np.asarray(edge_attr, np.float32).reshape(-1)
    deg = np.bincount(dst, minlength=N)

    pid_of = np.empty(N, np.int64)
    deg_sorted = np.zeros((C, NPCP), np.int64)
    for c in range(C):
        lo = c * NPC
        order = np.argsort(-deg[lo:lo + NPC], kind="stable") + lo
        pid_of[order] = c * NPCP + np.arange(NPC)
        deg_sorted[c, :NPC] = deg[order]

    # per-chunk padded degree (max over cores; rows are degree-sorted desc,
    # so the first row of a chunk is its max)
    Dpad = np.zeros(KCH, np.int64)
    for k in range(KCH):
        m = int(deg_sorted[:, k * 128].max())
        Dpad[k] = max(4, ((m + 3) // 4) * 4)
    offs = np.zeros(KCH + 1, np.int64)
    offs[1:] = np.cumsum(Dpad)
    S = int(offs[-1])

    # slot fill (fully vectorized)
    e_ord = np.argsort(dst, kind="stable")
    dst_s, src_s, ea_s = dst[e_ord], src[e_ord], ea[e_ord]
    startn = np.zeros(N + 1, np.int64)
    startn[1:] = np.cumsum(deg)
    j = np.arange(E) - startn[dst_s]
    gpid = pid_of[dst_s]
    c_a = gpid // NPCP
    loc = gpid % NPCP
    k_a = loc // 128
    r_a = loc % 128
    col = offs[k_a] + j
    srcs = np.zeros((C, 128, S), np.int32)
    eas = np.zeros((C, 128, S), np.float32)
    lbs = np.full((C, 128, S), -1e9, np.float32)
    srcs[c_a, r_a, col] = pid_of[src_s].astype(np.int32)
    eas[c_a, r_a, col] = ea_s
    lbs[c_a, r_a, col] = 0.0

    # transposed, permuted node features [C, 64, NPCP]
    x_P = np.zeros((cfg.NT, D), np.float32)
    x_P[pid_of] = np.asarray(x, np.float32)
    xT = np.ascontiguousarray(x_P.reshape(C, NPCP, D).transpose(0, 2, 1))

    # xr gather ids per core [128, KCH]
    tids = np.zeros((C, 128, KCH), np.int32)
    r = np.arange(128)
    for c in range(C):
        for k in range(KCH):
            tids[c, :, k] = c * NPCP + k * 128 + r

    # pooling indicator [C, 128, KCH*G64]; G padded to 64 cols for matmul
    GP = 64
    pool = np.zeros((cfg.NT, GP), np.float32)
    pool[pid_of, np.asarray(batch, np.int64)] = 1.0
    pind = np.ascontiguousarray(
        pool.reshape(C, KCH, 128, GP).transpose(0, 2, 1, 3).reshape(C, 128, KCH * GP))

    wlr = np.stack([np.hstack([weights[f"Wl{l}"], weights[f"Wr{l}"]]).astype(np.float32)
                    for l in (1, 2, 3)])                      # [3, 64, 128]
    wer = np.stack([np.repeat(weights[f"We{l}"].astype(np.float32), 128, axis=0)
                    for l in (1, 2, 3)])                      # [3, 128, 64]
    attr = np.stack([np.tile(weights[f"att{l}"].astype(np.float32), (128, 1))
                     for l in (1, 2, 3)])                     # [3, 128, 64]
    # biases (zero in the reference init, but handle generally)
    blbr = np.stack([np.tile(np.concatenate([weights[f"bl{l}"], weights[f"br{l}"]]).astype(np.float32), (128, 1))
                     for l in (1, 2, 3)])                     # [3, 128, 128]
    brep = np.stack([np.tile(weights[f"b{l}"].astype(np.float32), (128, 1))
                     for l in (1, 2, 3)])                     # [3, 128, 64]
    has_blbr = bool(np.any(blbr))
    has_b = bool(np.any(brep))

    counts = np.bincount(np.asarray(batch, np.int64), minlength=cfg.G).astype(np.float32)
    assert deg.min() >= 1, "isolated real node: unsupported fast path"

    meta = dict(Dpad=Dpad, offs=offs, S=S, has_blbr=has_blbr, has_b=has_b)
    percore = [dict(srcs=srcs[c], eas=eas[c], lbs=lbs[c], tids=tids[c], pind=pind[c],
                    xT=xT, wlr=wlr, wer=wer, attr=attr, blbr=blbr, brep=brep)
               for c in range(C)]
    return meta, percore, counts


def build_program(cfg, meta, dbg=False):
    Dpad, offs, S = meta["Dpad"], meta["offs"], meta["S"]
    C, D, KCH, NPCP, NT = cfg.C, cfg.D, cfg.KCH, cfg.NPCP, cfg.NT
    nc = bacc.Bacc("TRN2", target_bir_lowering=False, debug=False,
                   enable_asserts=False, num_devices=C)

    xT_h = nc.dram_tensor("xT", [C, D, NPCP], F32, kind="ExternalInput")
    srcs_h = nc.dram_tensor("srcs", [128, S], I32, kind="ExternalInput")
    eas_h = nc.dram_tensor("eas", [128, S], F32, kind="ExternalInput")
    lbs_h = nc.dram_tensor("lbs", [128, S], F32, kind="ExternalInput")
    tids_h = nc.dram_tensor("tids", [128, KCH], I32, kind="ExternalInput")
    pind_h = nc.dram_tensor("pind", [128, KCH * 64], F32, kind="ExternalInput")
    wlr_h = nc.dram_tensor("wlr", [3, 64, 128], F32, kind="ExternalInput")
    wer_h = nc.dram_tensor("wer", [3, 128, 64], F32, kind="ExternalInput")
    attr_h = nc.dram_tensor("attr", [3, 128, 64], F32, kind="ExternalInput")
    blbr_h = nc.dram_tensor("blbr", [3, 128, 128], F32, kind="ExternalInput")
    brep_h = nc.dram_tensor("brep", [3, 128, 64], F32, kind="ExternalInput")
    pout_h = nc.dram_tensor("pool_part", [64, 64], F32, kind="ExternalOutput")
    if dbg:
        xl_d = nc.dram_tensor("xl_dump", [NT, D], F32, kind="ExternalOutput")
        hTP_d = nc.dram_tensor("hTP_dump", [C, D, NPCP], F32, kind="ExternalOutput")
        g_d = nc.dram_tensor("g_dump", [128, int(Dpad[0]) * 64], F32, kind="ExternalOutput")

    xl_h = nc.dram_tensor("xl_scr", [NT, D], F32, kind="Internal")
    xr_h = nc.dram_tensor("xr_scr", [NT, D], F32, kind="Internal")
    hsh_h = nc.dram_tensor("hsh", [D, NPCP], F32, kind="Internal")
    hTP_h = nc.dram_tensor("hTP", [C, D, NPCP], F32, kind="Internal",
                           addr_space="Shared")

    has_blbr, has_b = meta["has_blbr"], meta["has_b"]

    with ExitStack() as ctx:
        tc = ctx.enter_context(tile.TileContext(nc))
        cpool = ctx.enter_context(tc.tile_pool(name="const", bufs=1))
        tpool = ctx.enter_context(tc.tile_pool(name="lhsT", bufs=2))
        xpool = ctx.enter_context(tc.tile_pool(name="xx", bufs=2))
        gpool = ctx.enter_context(tc.tile_pool(name="edge_g", bufs=2))
        t1pool = ctx.enter_context(tc.tile_pool(name="edge_t1", bufs=2))
        t2pool = ctx.enter_context(tc.tile_pool(name="edge_t2", bufs=2))
        spool = ctx.enter_context(tc.tile_pool(name="small", bufs=8))
        opool = ctx.enter_context(tc.tile_pool(name="out", bufs=4))
        mmpool = ctx.enter_context(tc.tile_pool(name="psum_mm", bufs=2, space="PSUM"))
        tppool = ctx.enter_context(tc.tile_pool(name="psum_tp", bufs=2, space="PSUM"))
        pppool = ctx.enter_context(tc.tile_pool(name="psum_pool", bufs=1, space="PSUM"))

        # resident SBUF
        ident = cpool.tile([128, 128], F32)
        make_identity(nc, ident[:])
        srcs_sb = cpool.tile([128, S], I32)
        nc.sync.dma_start(srcs_sb[:], srcs_h.ap())
        eas_sb = cpool.tile([128, S], F32)
        nc.sync.dma_start(eas_sb[:], eas_h.ap())
        lbs_sb = cpool.tile([128, S], F32)
        nc.sync.dma_start(lbs_sb[:], lbs_h.ap())
        tids_sb = cpool.tile([128, KCH], I32)
        nc.sync.dma_start(tids_sb[:], tids_h.ap())
        pind_sb = cpool.tile([128, KCH * 64], F32)
        nc.sync.dma_start(pind_sb[:], pind_h.ap())
        wlr_sb, wer_sb, attr_sb, blbr_sb, brep_sb = [], [], [], [], []
        for l in range(3):
            w1 = cpool.tile([64, 128], F32)
            nc.sync.dma_start(w1[:], wlr_h.ap()[l])
            wlr_sb.append(w1)
            w2 = cpool.tile([128, 64], F32)
            nc.sync.dma_start(w2[:], wer_h.ap()[l])
            wer_sb.append(w2)
            w3 = cpool.tile([128, 64], F32)
            nc.sync.dma_start(w3[:], attr_h.ap()[l])
            attr_sb.append(w3)
            if has_blbr:
                w4 = cpool.tile([128, 128], F32)
                nc.sync.dma_start(w4[:], blbr_h.ap()[l])
                blbr_sb.append(w4)
            if has_b:
                w5 = cpool.tile([128, 64], F32)
                nc.sync.dma_start(w5[:], brep_h.ap()[l])
                brep_sb.append(w5)

        packs = [4] * (KCH // 4) + ([KCH % 4] if KCH % 4 else [])

        pp = None
        for l in range(3):
            src3d = xT_h.ap() if l == 0 else hTP_h.ap()
            # ---- transform: xl/xr = h @ [Wl|Wr] for ALL nodes (replicated)
            for sec in range(C):
                t0 = 0
                for gsz in packs:
                    lh = tpool.tile([64, gsz * 128], F32)
                    nc.sync.dma_start(lh[:], src3d[sec, :, t0 * 128:(t0 + gsz) * 128])
                    ps = mmpool.tile([128, gsz * 128], F32)
                    for a in range(gsz):
                        nc.tensor.matmul(out=ps[:, a * 128:(a + 1) * 128],
                                         lhsT=lh[:, a * 128:(a + 1) * 128],
                                         rhs=wlr_sb[l][:], start=True, stop=True)
                    xx = xpool.tile([128, gsz * 128], F32)
                    if has_blbr:
                        bb = blbr_sb[l][:].unsqueeze(1).to_broadcast([128, gsz, 128])
                        nc.vector.tensor_tensor(
                            out=xx[:].rearrange("p (a q) -> p a q", q=128),
                            in0=ps[:].rearrange("p (a q) -> p a q", q=128),
                            in1=bb, op=ALU.add)
                    else:
                        nc.scalar.activation(out=xx[:], in_=ps[:], func=ACT.Copy)
                    row0 = sec * NPCP + t0 * 128
                    xx3 = xx[:].rearrange("p (a q) -> p a q", q=128)
                    dst_l = xl_h.ap()[row0:row0 + gsz * 128, :].rearrange(
                        "(a p) d -> p a d", p=128)
                    dst_r = xr_h.ap()[row0:row0 + gsz * 128, :].rearrange(
                        "(a p) d -> p a d", p=128)
                    nc.sync.dma_start(dst_l, xx3[:, :, 0:64])
                    nc.sync.dma_start(dst_r, xx3[:, :, 64:128])
                    t0 += gsz

            # ---- edge stage over chunks of 128 dst nodes
            if l == 2:
                pp = pppool.tile([64, 64], F32)
            for k in range(KCH):
                Dk = int(Dpad[k])
                off = int(offs[k])
                g = gpool.tile([128, Dk * 64], F32)
                g3 = g[:].rearrange("p (s d) -> p s d", d=64)
                nc.gpsimd.indirect_dma_start(
                    out=g[:], out_offset=None, in_=xl_h.ap(),
                    in_offset=IndirectOffsetOnAxis(
                        ap=srcs_sb[:, off:off + Dk], axis=0))
                if dbg and l == 0 and k == 0:
                    nc.sync.dma_start(g_d.ap(), g[:])
                xr_t = opool.tile([128, 64], F32)
                nc.gpsimd.indirect_dma_start(
                    out=xr_t[:], out_offset=None, in_=xr_h.ap(),
                    in_offset=IndirectOffsetOnAxis(ap=tids_sb[:, k:k + 1], axis=0))

                t1 = t1pool.tile([128, Dk * 64], F32)
                t1v = t1[:].rearrange("p (s d) -> p s d", d=64)
                eav = eas_sb[:, off:off + Dk].unsqueeze(2).to_broadcast([128, Dk, 64])
                wv = wer_sb[l][:].unsqueeze(1).to_broadcast([128, Dk, 64])
                nc.vector.tensor_tensor(out=t1v, in0=eav, in1=wv, op=ALU.mult)
                nc.vector.tensor_tensor(out=t1v, in0=t1v, in1=g3, op=ALU.add)
                xv = xr_t[:].unsqueeze(1).to_broadcast([128, Dk, 64])
                nc.vector.tensor_tensor(out=t1v, in0=t1v, in1=xv, op=ALU.add)
                # leaky relu: max(z, 0.2 z)
                t2 = t2pool.tile([128, Dk * 64], F32)
                t2v = t2[:].rearrange("p (s d) -> p s d", d=64)
                nc.scalar.activation(out=t2[:], in_=t1[:], func=ACT.Copy, scale=0.2)
                nc.vector.tensor_tensor(out=t2[:], in0=t2[:], in1=t1[:], op=ALU.max)
                av = attr_sb[l][:].unsqueeze(1).to_broadcast([128, Dk, 64])
                nc.vector.tensor_tensor(out=t1v, in0=t2v, in1=av, op=ALU.mult)
                lg = spool.tile([128, Dk], F32)
                nc.vector.tensor_reduce(out=lg[:], in_=t1v, axis=AX.X, op=ALU.add)
                nc.vector.tensor_tensor(out=lg[:], in0=lg[:],
                                        in1=lbs_sb[:, off:off + Dk], op=ALU.add)
                mn = spool.tile([128, 1], F32)
                nc.vector.tensor_reduce(out=mn[:], in_=lg[:], axis=AX.X, op=ALU.max,
                                        negate=True)
                pt = spool.tile([128, Dk], F32)
                nc.scalar.activation(out=pt[:], in_=lg[:], func=ACT.Exp,
                                     bias=mn[:, 0:1], scale=1.0)
                st = spool.tile([128, 1], F32)
                nc.vector.tensor_reduce(out=st[:], in_=pt[:], axis=AX.X, op=ALU.add)
                ri = spool.tile([128, 1], F32)
                nc.vector.reciprocal(ri[:], st[:])
                pv = pt[:].unsqueeze(2).to_broadcast([128, Dk, 64])
                nc.vector.tensor_tensor(out=t2v, in0=g3, in1=pv, op=ALU.mult)
                cur = Dk
                while cur > 1:
                    h = cur // 2
                    nc.vector.tensor_tensor(out=t2v[:, 0:h, :], in0=t2v[:, 0:h, :],
                                            in1=t2v[:, h:2 * h, :], op=ALU.add)
                    if cur % 2:
                        nc.vector.tensor_tensor(out=t2v[:, 0:1, :], in0=t2v[:, 0:1, :],
                                                in1=t2v[:, cur - 1:cur, :], op=ALU.add)
                    cur = h
                ot = opool.tile([128, 64], F32)
                riv = ri[:].to_broadcast([128, 64])
                nc.vector.tensor_tensor(out=ot[:], in0=t2[:, 0:64], in1=riv,
                                        op=ALU.mult)
                if has_b:
                    nc.vector.tensor_tensor(out=ot[:], in0=ot[:], in1=brep_sb[l][:],
                                            op=ALU.add)
                if l < 2:
                    ot2 = opool.tile([128, 64], F32)
                    nc.scalar.activation(out=ot2[:], in_=ot[:], func=ACT.Relu)
                    tp = tppool.tile([64, 128], F32)
                    nc.tensor.transpose(out=tp[:], in_=ot2[:], identity=ident[:])
                    hT_t = opool.tile([64, 128], F32)
                    nc.scalar.activation(out=hT_t[:], in_=tp[:], func=ACT.Copy)
                    nc.sync.dma_start(hsh_h.ap()[:, k * 128:(k + 1) * 128], hT_t[:])
                else:
                    nc.tensor.matmul(out=pp[:], lhsT=pind_sb[:, k * 64:(k + 1) * 64],
                                     rhs=ot[:], start=(k == 0), stop=(k == KCH - 1))
            if l < 2:
                nc.gpsimd.collective_compute(
                    "AllGather", ALU.bypass,
                    replica_groups=[list(range(C))],
                    ins=[hsh_h.ap().opt()], outs=[hTP_h.ap().opt()])
            if dbg and l == 0:
                nc.sync.dma_start(xl_d.ap(), xl_h.ap())
                nc.sync.dma_start(hTP_d.ap(), hTP_h.ap())

        po = opool.tile([64, 64], F32)
        nc.vector.tensor_copy(po[:], pp[:])
        nc.sync.dma_start(pout_h.ap(), po[:])

    nc.compile()
    return nc


_CACHE = {}


def _get_weights(inputs):
    keys = []
    for l in (1, 2, 3):
        keys += [f"Wl{l}", f"bl{l}", f"Wr{l}", f"br{l}", f"We{l}", f"att{l}", f"b{l}"]
    return {k: np.asarray(inputs[k], np.float32) for k in keys}


def run_raw(inputs, trace=False):
    from concourse import bass_utils

    x = np.asarray(inputs["x"], np.float32)
    edge_index = np.asarray(inputs["edge_index"])
    edge_attr = np.asarray(inputs["edge_attr"], np.float32)
    batch = np.asarray(inputs["batch"])
    N, E = x.shape[0], edge_index.shape[1]
    G = 64
    cfg = Cfg(N=N, E=E, G=G)
    weights = _get_weights(inputs)

    meta, percore, counts = host_prep(cfg, x, edge_index, edge_attr, batch, weights)

    key = (N, E, int(meta["S"]), tuple(int(d) for d in meta["Dpad"]),
           meta["has_blbr"], meta["has_b"])
    if key not in _CACHE:
        _CACHE[key] = build_program(cfg, meta)
    nc = _CACHE[key]

    in_maps = [dict(pc) for pc in percore]
    res = bass_utils.run_bass_kernel_spmd(nc, in_maps, core_ids=list(range(cfg.C)),
                                          trace=trace)
    parts = np.zeros((64, 64), np.float64)
    for c in range(cfg.C):
        parts += np.asarray(res.results[c]["pool_part"], np.float64)
    hG = parts[:G, :cfg.D] / np.maximum(counts, 1.0)[:, None]
    Wlin = np.asarray(inputs["Wlin"], np.float64)
    blin = np.asarray(inputs["blin"], np.float64)
    return (hG @ Wlin + blin).astype(np.float32), res


def kernel(**inputs):
    out, _ = run_raw(inputs, trace=False)
    return out



# revision 9
# speedup vs baseline: 4.3066x; 2.8650x over previous
"""GATv2 (3 layers, heads=1) + global mean pool + linear on 8 Trainium2 cores.

Sharding: edges partitioned by dst-node range (8 x 6250 nodes). Per core,
local nodes are degree-sorted into 49 chunks of 128 rows; each chunk is
padded to a uniform per-chunk degree so segmented softmax / scatter-add
become dense row-wise ops. Node feature transforms (h@Wl, h@Wr) are
replicated on every core; per-edge source features come from an indirect
(gather) DMA out of DRAM. h is carried transposed ([64, nodes]) between
layers via AllGather so the next layer's matmuls need no on-device
transposes of the full node set.
"""

import os
import sys
from contextlib import ExitStack
from dataclasses import dataclass

import numpy as np

for _p in ("/opt/trn_rl_repo", "/root/.axon_site/_ro/trn_rl_repo"):
    if os.path.isdir(_p) and _p not in sys.path:
        sys.path.insert(0, _p)

import concourse.bass as bass
import concourse.tile as tile
from concourse import bacc, mybir
from concourse.bass import IndirectOffsetOnAxis
from concourse.masks import make_identity

F32 = mybir.dt.float32
BF16 = mybir.dt.bfloat16
I32 = mybir.dt.int32
ALU = mybir.AluOpType
ACT = mybir.ActivationFunctionType
AX = mybir.AxisListType


@dataclass
class Cfg:
    N: int          # real nodes
    E: int
    G: int          # graphs
    C: int = 8      # cores
    D: int = 64     # feature dim
    NPC: int = 0    # real nodes per core
    KCH: int = 0    # chunks of 128 rows per core
    NPCP: int = 0   # padded nodes per core = KCH*128
    NT: int = 0     # C*NPCP

    def __post_init__(self):
        self.NPC = self.N // self.C
        self.KCH = (self.NPC + 127) // 128
        self.NPCP = self.KCH * 128
        self.NT = self.C * self.NPCP


def host_prep(cfg, x, edge_index, edge_attr, batch, weights):
    C, NPC, NPCP, KCH, D = cfg.C, cfg.NPC, cfg.NPCP, cfg.KCH, cfg.D
    N, E = cfg.N, cfg.E
    src = np.asarray(edge_index[0], np.int64)
    dst = np.asarray(edge_index[1], np.int64)
    ea = np.asarray(edge_attr, np.float32).reshape(-1)
    deg = np.bincount(dst, minlength=N)

    pid_of = np.empty(N, np.int64)
    deg_sorted = np.zeros((C, NPCP), np.int64)
    for c in range(C):
        lo = c * NPC
        order = np.argsort(-deg[lo:lo + NPC], kind="stable") + lo
        pid_of[order] = c * NPCP + np.arange(NPC)
        deg_sorted[c, :NPC] = deg[order]

    # per-chunk padded degree (max over cores; rows are degree-sorted desc,
    # so the first row of a chunk is its max)
    Dpad = np.zeros(KCH, np.int64)
    for k in range(KCH):
        m = int(deg_sorted[:, k * 128].max())
        Dpad[k] = max(4, ((m + 3) // 4) * 4)
    offs = np.zeros(KCH + 1, np.int64)
    offs[1:] = np.cumsum(Dpad)
    S = int(offs[-1])

    # slot fill (fully vectorized)
    e_ord = np.argsort(dst, kind="stable")
    dst_s, src_s, ea_s = dst[e_ord], src[e_ord], ea[e_ord]
    startn = np.zeros(N + 1, np.int64)
    startn[1:] = np.cumsum(deg)
    j = np.arange(E) - startn[dst_s]
    gpid = pid_of[dst_s]
    c_a = gpid // NPCP
    loc = gpid % NPCP
    k_a = loc // 128
    r_a = loc % 128
    col = offs[k_a] + j
    srcs = np.zeros((C, 128, S), np.int32)
    eas = np.zeros((C, 128, S), np.float32)
    lbs = np.full((C, 128, S), -1e9, np.float32)
    srcs[c_a, r_a, col] = pid_of[src_s].astype(np.int32)
    eas[c_a, r_a, col] = ea_s
    lbs[c_a, r_a, col] = 0.0

    # transposed, permuted node features [C, 64, NPCP]
    x_P = np.zeros((cfg.NT, D), np.float32)
    x_P[pid_of] = np.asarray(x, np.float32)
    xT = np.ascontiguousarray(x_P.reshape(C, NPCP, D).transpose(0, 2, 1))

    # xr gather ids per core [128, KCH]
    tids = np.zeros((C, 128, KCH), np.int32)
    r = np.arange(128)
    for c in range(C):
        for k in range(KCH):
            tids[c, :, k] = c * NPCP + k * 128 + r

    # pooling indicator [C, 128, KCH*G64]; G padded to 64 cols for matmul
    GP = 64
    pool = np.zeros((cfg.NT, GP), np.float32)
    pool[pid_of, np.asarray(batch, np.int64)] = 1.0
    pind = np.ascontiguousarray(
        pool.reshape(C, KCH, 128, GP).transpose(0, 2, 1, 3).reshape(C, 128, KCH * GP))

    wlr = np.stack([np.hstack([weights[f"Wl{l}"], weights[f"Wr{l}"]]).astype(np.float32)
                    for l in (1, 2, 3)])                      # [3, 64, 128]
    wer = np.stack([np.repeat(weights[f"We{l}"].astype(np.float32), 128, axis=0)
                    for l in (1, 2, 3)])                      # [3, 128, 64]
    attr = np.stack([np.tile(weights[f"att{l}"].astype(np.float32), (128, 1))
                     for l in (1, 2, 3)])                     # [3, 128, 64]
    # biases (zero in the reference init, but handle generally)
    blbr = np.stack([np.tile(np.concatenate([weights[f"bl{l}"], weights[f"br{l}"]]).astype(np.float32), (128, 1))
                     for l in (1, 2, 3)])                     # [3, 128, 128]
    brep = np.stack([np.tile(weights[f"b{l}"].astype(np.float32), (128, 1))
                     for l in (1, 2, 3)])                     # [3, 128, 64]
    has_blbr = bool(np.any(blbr))
    has_b = bool(np.any(brep))

    counts = np.bincount(np.asarray(batch, np.int64), minlength=cfg.G).astype(np.float32)
    assert deg.min() >= 1, "isolated real node: unsupported fast path"

    import ml_dtypes
    xT16 = xT.astype(ml_dtypes.bfloat16)
    pind16 = pind.astype(ml_dtypes.bfloat16)
    wlr16 = wlr.astype(ml_dtypes.bfloat16)
    meta = dict(Dpad=Dpad, offs=offs, S=S, has_blbr=has_blbr, has_b=has_b)
    percore = []
    for c in range(C):
        pc = dict(srcs=srcs[c], eas=eas[c], lbs=lbs[c], tids=tids[c],
                  pind=pind16[c], xT=xT16, wlr=wlr16, wer=wer, attr=attr)
        if has_blbr:
            pc["blbr"] = blbr
        if has_b:
            pc["brep"] = brep
        percore.append(pc)
    return meta, percore, counts


def build_program(cfg, meta, dbg=False):
    Dpad, offs, S = meta["Dpad"], meta["offs"], meta["S"]
    C, D, KCH, NPCP, NT = cfg.C, cfg.D, cfg.KCH, cfg.NPCP, cfg.NT
    nc = bacc.Bacc("TRN2", target_bir_lowering=False, debug=False,
                   enable_asserts=False, num_devices=C)

    xT_h = nc.dram_tensor("xT", [C, D, NPCP], BF16, kind="ExternalInput")
    srcs_h = nc.dram_tensor("srcs", [128, S], I32, kind="ExternalInput")
    eas_h = nc.dram_tensor("eas", [128, S], F32, kind="ExternalInput")
    lbs_h = nc.dram_tensor("lbs", [128, S], F32, kind="ExternalInput")
    tids_h = nc.dram_tensor("tids", [128, KCH], I32, kind="ExternalInput")
    pind_h = nc.dram_tensor("pind", [128, KCH * 64], BF16, kind="ExternalInput")
    wlr_h = nc.dram_tensor("wlr", [3, 64, 128], BF16, kind="ExternalInput")
    wer_h = nc.dram_tensor("wer", [3, 128, 64], F32, kind="ExternalInput")
    attr_h = nc.dram_tensor("attr", [3, 128, 64], F32, kind="ExternalInput")
    has_blbr, has_b = meta["has_blbr"], meta["has_b"]
    if has_blbr:
        blbr_h = nc.dram_tensor("blbr", [3, 128, 128], F32, kind="ExternalInput")
    if has_b:
        brep_h = nc.dram_tensor("brep", [3, 128, 64], F32, kind="ExternalInput")
    pout_h = nc.dram_tensor("pool_part", [64, 64], F32, kind="ExternalOutput")
    if dbg:
        xl_d = nc.dram_tensor("xl_dump", [NT, D], F32, kind="ExternalOutput")
        hTP_d = nc.dram_tensor("hTP_dump", [C, D, NPCP], F32, kind="ExternalOutput")
        g_d = nc.dram_tensor("g_dump", [128, int(Dpad[0]) * 64], F32, kind="ExternalOutput")

    xl_h = nc.dram_tensor("xl_scr", [NT, D], F32, kind="Internal")
    xr_h = nc.dram_tensor("xr_scr", [NT, D], F32, kind="Internal")
    hsh_h = nc.dram_tensor("hsh", [D, NPCP], BF16, kind="Internal")
    hTP_h = nc.dram_tensor("hTP", [C, D, NPCP], BF16, kind="Internal",
                           addr_space="Shared")

    with ExitStack() as ctx:
        tc = ctx.enter_context(tile.TileContext(nc))
        cpool = ctx.enter_context(tc.tile_pool(name="const", bufs=1))
        tpool = ctx.enter_context(tc.tile_pool(name="lhsT", bufs=2))
        xpool = ctx.enter_context(tc.tile_pool(name="xx", bufs=2))
        gpool = ctx.enter_context(tc.tile_pool(name="edge_g", bufs=2))
        t1pool = ctx.enter_context(tc.tile_pool(name="edge_t1", bufs=2))
        t2pool = ctx.enter_context(tc.tile_pool(name="edge_t2", bufs=2))
        spool = ctx.enter_context(tc.tile_pool(name="small", bufs=8))
        opool = ctx.enter_context(tc.tile_pool(name="out", bufs=4))
        mmpool = ctx.enter_context(tc.tile_pool(name="psum_mm", bufs=2, space="PSUM"))
        tppool = ctx.enter_context(tc.tile_pool(name="psum_tp", bufs=2, space="PSUM"))
        pppool = ctx.enter_context(tc.tile_pool(name="psum_pool", bufs=1, space="PSUM"))

        # resident SBUF
        ident = cpool.tile([128, 128], F32)
        make_identity(nc, ident[:])
        srcs_sb = cpool.tile([128, S], I32)
        nc.sync.dma_start(srcs_sb[:], srcs_h.ap())
        eas_sb = cpool.tile([128, S], F32)
        nc.sync.dma_start(eas_sb[:], eas_h.ap())
        lbs_sb = cpool.tile([128, S], F32)
        nc.sync.dma_start(lbs_sb[:], lbs_h.ap())
        tids_sb = cpool.tile([128, KCH], I32)
        nc.sync.dma_start(tids_sb[:], tids_h.ap())
        pind_sb = cpool.tile([128, KCH * 64], BF16)
        nc.sync.dma_start(pind_sb[:], pind_h.ap())
        wlr_sb, wer_sb, attr_sb, blbr_sb, brep_sb = [], [], [], [], []
        for l in range(3):
            w1 = cpool.tile([64, 128], BF16)
            nc.sync.dma_start(w1[:], wlr_h.ap()[l])
            wlr_sb.append(w1)
            w2 = cpool.tile([128, 64], F32)
            nc.sync.dma_start(w2[:], wer_h.ap()[l])
            wer_sb.append(w2)
            w3 = cpool.tile([128, 64], F32)
            nc.sync.dma_start(w3[:], attr_h.ap()[l])
            attr_sb.append(w3)
            if has_blbr:
                w4 = cpool.tile([128, 128], F32)
                nc.sync.dma_start(w4[:], blbr_h.ap()[l])
                blbr_sb.append(w4)
            if has_b:
                w5 = cpool.tile([128, 64], F32)
                nc.sync.dma_start(w5[:], brep_h.ap()[l])
                brep_sb.append(w5)

        packs = [4] * (KCH // 4) + ([KCH % 4] if KCH % 4 else [])

        pp = None
        for l in range(3):
            src3d = xT_h.ap() if l == 0 else hTP_h.ap()
            # ---- transform: xl/xr = h @ [Wl|Wr] for ALL nodes (replicated)
            for sec in range(C):
                t0 = 0
                for gsz in packs:
                    lh = tpool.tile([64, gsz * 128], BF16)
                    nc.sync.dma_start(lh[:], src3d[sec, :, t0 * 128:(t0 + gsz) * 128])
                    ps = mmpool.tile([128, gsz * 128], F32)
                    for a in range(gsz):
                        nc.tensor.matmul(out=ps[:, a * 128:(a + 1) * 128],
                                         lhsT=lh[:, a * 128:(a + 1) * 128],
                                         rhs=wlr_sb[l][:], start=True, stop=True)
                    xx = xpool.tile([128, gsz * 128], F32)
                    if has_blbr:
                        bb = blbr_sb[l][:].unsqueeze(1).to_broadcast([128, gsz, 128])
                        nc.vector.tensor_tensor(
                            out=xx[:].rearrange("p (a q) -> p a q", q=128),
                            in0=ps[:].rearrange("p (a q) -> p a q", q=128),
                            in1=bb, op=ALU.add)
                    else:
                        nc.scalar.activation(out=xx[:], in_=ps[:], func=ACT.Copy)
                    row0 = sec * NPCP + t0 * 128
                    xx3 = xx[:].rearrange("p (a q) -> p a q", q=128)
                    dst_l = xl_h.ap()[row0:row0 + gsz * 128, :].rearrange(
                        "(a p) d -> p a d", p=128)
                    dst_r = xr_h.ap()[row0:row0 + gsz * 128, :].rearrange(
                        "(a p) d -> p a d", p=128)
                    nc.sync.dma_start(dst_l, xx3[:, :, 0:64])
                    nc.sync.dma_start(dst_r, xx3[:, :, 64:128])
                    t0 += gsz

            # ---- edge stage over chunks of 128 dst nodes
            if l == 2:
                pp = pppool.tile([64, 64], F32)
            for k in range(KCH):
                Dk = int(Dpad[k])
                off = int(offs[k])
                g = gpool.tile([128, Dk * 64], F32)
                g3 = g[:].rearrange("p (s d) -> p s d", d=64)
                for s in range(Dk):
                    nc.gpsimd.indirect_dma_start(
                        out=g[:, s * 64:(s + 1) * 64], out_offset=None, in_=xl_h.ap(),
                        in_offset=IndirectOffsetOnAxis(
                            ap=srcs_sb[:, off + s:off + s + 1], axis=0))
                if dbg and l == 0 and k == 0:
                    nc.sync.dma_start(g_d.ap(), g[:])
                xr_t = opool.tile([128, 64], F32)
                nc.gpsimd.indirect_dma_start(
                    out=xr_t[:], out_offset=None, in_=xr_h.ap(),
                    in_offset=IndirectOffsetOnAxis(ap=tids_sb[:, k:k + 1], axis=0))

                t1 = t1pool.tile([128, Dk * 64], F32)
                t1v = t1[:].rearrange("p (s d) -> p s d", d=64)
                eav = eas_sb[:, off:off + Dk].unsqueeze(2).to_broadcast([128, Dk, 64])
                wv = wer_sb[l][:].unsqueeze(1).to_broadcast([128, Dk, 64])
                nc.vector.tensor_tensor(out=t1v, in0=eav, in1=wv, op=ALU.mult)
                nc.vector.tensor_tensor(out=t1v, in0=t1v, in1=g3, op=ALU.add)
                xv = xr_t[:].unsqueeze(1).to_broadcast([128, Dk, 64])
                nc.vector.tensor_tensor(out=t1v, in0=t1v, in1=xv, op=ALU.add)
                # leaky relu: max(z, 0.2 z)
                t2 = t2pool.tile([128, Dk * 64], F32)
                t2v = t2[:].rearrange("p (s d) -> p s d", d=64)
                nc.scalar.activation(out=t2[:], in_=t1[:], func=ACT.Copy, scale=0.2)
                nc.vector.tensor_tensor(out=t2[:], in0=t2[:], in1=t1[:], op=ALU.max)
                av = attr_sb[l][:].unsqueeze(1).to_broadcast([128, Dk, 64])
                nc.vector.tensor_tensor(out=t1v, in0=t2v, in1=av, op=ALU.mult)
                lg = spool.tile([128, Dk], F32)
                nc.vector.tensor_reduce(out=lg[:], in_=t1v, axis=AX.X, op=ALU.add)
                nc.vector.tensor_tensor(out=lg[:], in0=lg[:],
                                        in1=lbs_sb[:, off:off + Dk], op=ALU.add)
                mn = spool.tile([128, 1], F32)
                nc.vector.tensor_reduce(out=mn[:], in_=lg[:], axis=AX.X, op=ALU.max,
                                        negate=True)
                pt = spool.tile([128, Dk], F32)
                nc.scalar.activation(out=pt[:], in_=lg[:], func=ACT.Exp,
                                     bias=mn[:, 0:1], scale=1.0)
                st = spool.tile([128, 1], F32)
                nc.vector.tensor_reduce(out=st[:], in_=pt[:], axis=AX.X, op=ALU.add)
                ri = spool.tile([128, 1], F32)
                nc.vector.reciprocal(ri[:], st[:])
                pv = pt[:].unsqueeze(2).to_broadcast([128, Dk, 64])
                nc.vector.tensor_tensor(out=t2v, in0=g3, in1=pv, op=ALU.mult)
                cur = Dk
                while cur > 1:
                    h = cur // 2
                    nc.vector.tensor_tensor(out=t2v[:, 0:h, :], in0=t2v[:, 0:h, :],
                                            in1=t2v[:, h:2 * h, :], op=ALU.add)
                    if cur % 2:
                        nc.vector.tensor_tensor(out=t2v[:, 0:1, :], in0=t2v[:, 0:1, :],
                                                in1=t2v[:, cur - 1:cur, :], op=ALU.add)
                    cur = h
                ot = opool.tile([128, 64], F32)
                riv = ri[:].to_broadcast([128, 64])
                nc.vector.tensor_tensor(out=ot[:], in0=t2[:, 0:64], in1=riv,
                                        op=ALU.mult)
                if has_b:
                    nc.vector.tensor_tensor(out=ot[:], in0=ot[:], in1=brep_sb[l][:],
                                            op=ALU.add)
                if l < 2:
                    ot2 = opool.tile([128, 64], F32)
                    nc.scalar.activation(out=ot2[:], in_=ot[:], func=ACT.Relu)
                    tp = tppool.tile([64, 128], F32)
                    nc.tensor.transpose(out=tp[:], in_=ot2[:], identity=ident[:])
                    hT_t = opool.tile([64, 128], BF16)
                    nc.scalar.activation(out=hT_t[:], in_=tp[:], func=ACT.Copy)
                    nc.sync.dma_start(hsh_h.ap()[:, k * 128:(k + 1) * 128], hT_t[:])
                else:
                    otb = opool.tile([128, 64], BF16)
                    nc.scalar.activation(out=otb[:], in_=ot[:], func=ACT.Copy)
                    nc.tensor.matmul(out=pp[:], lhsT=pind_sb[:, k * 64:(k + 1) * 64],
                                     rhs=otb[:], start=(k == 0), stop=(k == KCH - 1))
            if l < 2:
                nc.gpsimd.collective_compute(
                    "AllGather", ALU.bypass,
                    replica_groups=[list(range(C))],
                    ins=[hsh_h.ap().opt()], outs=[hTP_h.ap().opt()])
            if dbg and l == 0:
                nc.sync.dma_start(xl_d.ap(), xl_h.ap())
                nc.sync.dma_start(hTP_d.ap(), hTP_h.ap())

        po = opool.tile([64, 64], F32)
        nc.vector.tensor_copy(po[:], pp[:])
        nc.sync.dma_start(pout_h.ap(), po[:])

    nc.compile()
    return nc


_CACHE = {}


def _get_weights(inputs):
    keys = []
    for l in (1, 2, 3):
        keys += [f"Wl{l}", f"bl{l}", f"Wr{l}", f"br{l}", f"We{l}", f"att{l}", f"b{l}"]
    return {k: np.asarray(inputs[k], np.float32) for k in keys}


def run_raw(inputs, trace=False):
    from concourse import bass_utils

    x = np.asarray(inputs["x"], np.float32)
    edge_index = np.asarray(inputs["edge_index"])
    edge_attr = np.asarray(inputs["edge_attr"], np.float32)
    batch = np.asarray(inputs["batch"])
    N, E = x.shape[0], edge_index.shape[1]
    G = 64
    cfg = Cfg(N=N, E=E, G=G)
    weights = _get_weights(inputs)

    meta, percore, counts = host_prep(cfg, x, edge_index, edge_attr, batch, weights)

    key = (N, E, int(meta["S"]), tuple(int(d) for d in meta["Dpad"]),
           meta["has_blbr"], meta["has_b"])
    if key not in _CACHE:
        _CACHE[key] = build_program(cfg, meta)
    nc = _CACHE[key]

    in_maps = [dict(pc) for pc in percore]
    res = bass_utils.run_bass_kernel_spmd(nc, in_maps, core_ids=list(range(cfg.C)),
                                          trace=trace)
    parts = np.zeros((64, 64), np.float64)
    for c in range(cfg.C):
        parts += np.asarray(res.results[c]["pool_part"], np.float64)
    hG = parts[:G, :cfg.D] / np.maximum(counts, 1.0)[:, None]
    Wlin = np.asarray(inputs["Wlin"], np.float64)
    blin = np.asarray(inputs["blin"], np.float64)
    return (hG @ Wlin + blin).astype(np.float32), res


def kernel(**inputs):
    out, _ = run_raw(inputs, trace=False)
    return out

